# revision 23
# baseline (speedup 1.0000x reference)
"""Trainium2 Bass kernel for MaxCosineSimilarityBlock.

Reference computation (per batch b, channel c):
  windows  xw[t, s] = xpad[t + s]          (xpad = x padded by 31/32 zeros, S=64)
  xn[t, :] = xw[t, :] / max(||xw[t, :]||, 1e-8)
  sn[n, :] = shapelets[c, n, :] / max(||shapelets[c, n, :]||, 1e-8)
  out[b, c, t, n] = relu(xn[t, :] @ sn[n, :])

Shapes: x [32, 8, 1024] f32, shapelets [8, 512, 64] f32 -> out [32, 8, 1024, 512] f32.

Strategy: data-parallel over batch B across 8 cores (4 batches/core = 32
(b, c) rows/core).  Per row the conv-as-matmul runs on the PE with the
im2col window matrix streamed via an overlapping access pattern from a
fp16 copy of the padded rows staged in DRAM scratch (halves the 64x read
amplification of the im2col load).  Weight columns are strided by NT so
output partition p carries t = NT*p + j, which makes each partition's
output staging block 8KB-contiguous in HBM (fast stores).

The S=64 contraction only fills half the 128-row PE array, so rows are
processed in PAIRS mapped to the two 64-row PE tile groups
(tile_position (0,0) / (64,0)): row 2i's windows sit on SBUF partitions
0-63, row 2i+1's on partitions 64-127, the normalized+transposed
shapelets snT are mirrored across both partition halves once, and the two
matmuls per t-tile execute concurrently on the PE (~2x throughput at
K=64).

  lhsT = XwinT [S=64, 128 t]  (fp16 weights, FWL-accelerated self-load)
  rhs  = snT   [S=64, N=512]  (normalized shapelets, transposed+cast once)
  psum [128 t, 512 n] f32

PSUM eviction applies relu and the per-window inverse norm (per-partition
scale) while downcasting to the f16 output staging tile; evictions
alternate between the ACT and DVE engines (neither alone can keep up) and
are the steady-state governor, so prep is kept off their critical path:
shapelets arrive host-permuted for a contiguous load, their norm chain is
batched per channel-group on DVE, the window-norm log-tree runs on the
otherwise idle GPSIMD engine, and output stores ride the GPSIMD (SWDGE)
queue.
"""

import os
import sys

for _p in ("/opt/trn_rl_repo", "/root/.axon_site/_ro/trn_rl_repo"):
    if os.path.isdir(_p) and _p not in sys.path:
        sys.path.insert(0, _p)

import numpy as np

import concourse.bass as bass
import concourse.mybir as mybir
from concourse import masks, tile
from concourse.bass_utils import run_bass_kernel_spmd

F32 = mybir.dt.float32
F32R = mybir.dt.float32r
F16 = mybir.dt.float16
BF16 = mybir.dt.bfloat16
AF = mybir.ActivationFunctionType
ALU = mybir.AluOpType

B, C, T, S, N = 32, 8, 1024, 64, 512
NCORES = 8
PAD_L, PAD_R = (S - 1) // 2, (S - 1) // 2 + (S - 1) % 2  # 31, 32
TP = T + S - 1  # 1087
NT = T // 128  # 8 t-tiles per row
NK = N // 128  # 4 shapelet chunks per channel
NCK = C * NK  # 32 shapelet chunks total

# eviction cost model (ns, trace-fit) for load balancing ACT vs DVE
ACT_EVICT_NS = 737.0
DVE_EVICT_NS = 744.0

# window-norm chunking: each row's sliding sums computed as CH chunks of CL
CH, CL = 4, 256
CW = CL + S - 1  # 319 elements feed one chunk's sums


def build_nc(rows=B * C // NCORES, mm_dtype=F16, out_np_dtype=np.float16):
    """Build the per-core Bass program. `rows` = number of (b, c) rows."""
    out_dt = mybir.dt.from_np(np.dtype(out_np_dtype))
    use16 = mm_dtype in (F16, BF16)
    td = mm_dtype if use16 else F32  # transpose/snT dtype
    nc = bass.Bass("TRN2", target_bir_lowering=False, debug=False)
    xp = nc.dram_tensor("xp", [rows, TP], F32, kind="ExternalInput")
    # host sends shapelets pre-permuted: shp[p, i, s] = orig[(i*128 + p)//N,
    # (i*128 + p) % N, s] so a contiguous load puts chunk columns in order
    shp = nc.dram_tensor("shp", [128, NCK, S], F32, kind="ExternalInput")
    out = nc.dram_tensor("out", [rows, T, N], out_dt, kind="ExternalOutput")
    if use16:
        xp16 = nc.dram_tensor("xp16", [rows, TP], mm_dtype, kind="Internal")

    with tile.TileContext(nc) as tc:
        with (
            tc.tile_pool(name="const", bufs=1) as constp,
            tc.tile_pool(name="prep", bufs=2) as prep,
            tc.tile_pool(name="prep_ps", bufs=2, space="PSUM") as prep_ps,
            tc.tile_pool(name="xw", bufs=3) as xwp,
            tc.tile_pool(name="ostage", bufs=4) as ostagep,
            tc.tile_pool(name="mm_ps", bufs=6, space="PSUM") as mmps,
        ):
            ident = constp.tile([128, 128], F32)
            masks.make_identity(nc, ident[:])

            # ---- input loads (sync ring) + fp16 DRAM scratch staging ----
            if use16:
                xr = prep.tile([rows, TP], F32, tag="xr")
                nc.sync.dma_start(xr[:], xp.ap())
                xr16 = prep.tile([rows, TP], mm_dtype, tag="xr16")
                nc.vector.tensor_copy(xr16[:], xr[:])
                win_src, win_dt = xp16, mm_dtype
            else:
                win_src, win_dt = xp, F32
            sh_sb = prep.tile([128, NCK, S], F32, tag="shload")
            nc.sync.dma_start(sh_sb[:], shp.ap())
            # chunked overlapped rows for the 128-partition norm chain:
            # partition c*rows + r holds xpad[r, CL*c : CL*c + CW]
            xr2 = prep.tile([rows * CH, 320], F32, tag="xr2")
            nc.sync.dma_start(
                xr2[:, 0:CW], bass.AP(xp, 0, [[CL, CH], [TP, rows], [1, CW]])
            )
            if use16:
                nc.sync.dma_start(xp16.ap(), xr16[:])

            if use16:
                ident_t = constp.tile([128, 128], td)
                nc.scalar.copy(ident_t[:], ident[:])
            else:
                ident_t = ident

            # engine chooser: balance ACT vs DVE by accumulated busy-ns
            eng_ns = [0.0, 0.0]  # (ACT, DVE)

            def copy_ps(dst, src, fd):
                """PSUM->SBUF copy on the less-loaded of ACT/DVE."""
                act_c, dve_c = (172 + fd) / 1.2 + 100, (120 + fd) / 0.96 + 100
                if eng_ns[0] + act_c <= eng_ns[1] + dve_c:
                    nc.scalar.copy(dst, src)
                    eng_ns[0] += act_c
                else:
                    nc.vector.tensor_copy(dst, src)
                    eng_ns[1] += dve_c

            # ---- shapelet prep: per-group batched inverse norms (DVE), ----
            # ---- fp16 PE transposes into snT's lower partition half    ----
            snT = constp.tile([128, NCK * 128], td)
            GN, GS = 4, NCK // 4  # channel groups of 8 chunks
            sq = prep.tile([128, NCK * S], F32, tag="sq")
            ssq = prep.tile([128, NCK, 1], F32, tag="ssq")
            recs = prep.tile([128, NCK], F32, tag="recs")
            invs = prep.tile([128, NCK], F32, tag="invs")
            nrm16 = prep.tile([128, NCK, S], td, tag="nrm16")
            shv = sh_sb[:].rearrange("p k s -> p (k s)")
            ssqf = ssq[:].rearrange("p k 1 -> p (k 1)")
            for g in range(GN):
                ck0, ck1 = g * GS, (g + 1) * GS
                fsl = slice(ck0 * S, ck1 * S)
                nc.vector.tensor_mul(sq[:, fsl], shv[:, fsl], shv[:, fsl])
                nc.vector.reduce_sum(
                    ssq[:, ck0:ck1, :],
                    sq[:, fsl].rearrange("p (k s) -> p k s", s=S),
                    axis=mybir.AxisListType.X,
                )
                nc.vector.tensor_scalar_max(
                    ssqf[:, ck0:ck1], ssqf[:, ck0:ck1], 1e-16
                )
                nc.vector.reciprocal(recs[:, ck0:ck1], ssqf[:, ck0:ck1])
                nc.scalar.activation(
                    invs[:, ck0:ck1], recs[:, ck0:ck1], AF.Sqrt
                )
                nc.vector.tensor_mul(
                    nrm16[:, ck0:ck1, :],
                    sh_sb[:, ck0:ck1, :],
                    invs[:, ck0:ck1][:, :, None].broadcast_to([128, GS, S]),
                )
            tdw = 64 if use16 else 128  # f32-element width of a [64,128] td tile
            for ck in range(NCK):
                ps = prep_ps.tile([128, 512], F32, tag="ps")
                ps_t = ps[0:64, 0:tdw].bitcast(td)
                nc.tensor.transpose(ps_t, nrm16[:, ck, :], ident_t[:])
                copy_ps(snT[0:64, ck * 128 : (ck + 1) * 128], ps_t, 128)
            # mirror to partitions 64-127 for the second PE tile group
            nc.gpsimd.dma_start(snT[64:128, :], snT[0:64, :])

            # ---- window inverse norms (gpsimd log-tree, 128 partitions) ----
            cur = prep.tile([rows * CH, 320], F32, tag="wc0")
            nc.gpsimd.tensor_mul(cur[:, 0:CW], xr2[:, 0:CW], xr2[:, 0:CW])
            for k in range(6):
                off = 1 << k
                ln = CW - (2 * off - 1)
                nxt = prep.tile([rows * CH, 320], F32, tag=f"wc{k + 1}")
                nc.gpsimd.tensor_add(
                    nxt[:, 0:ln], cur[:, 0:ln], cur[:, off : off + ln]
                )
                cur = nxt
            # cur[:, 0:CL] = sliding sums; invw2[c*rows+r, q] = 1/||win(CL*c+q)||
            nc.gpsimd.tensor_scalar_max(cur[:, 0:CL], cur[:, 0:CL], 1e-16)
            rec2 = prep.tile([rows * CH, CL], F32, tag="rec2")
            nc.vector.reciprocal(rec2[:], cur[:, 0:CL])
            invw2 = prep.tile([rows * CH, CL], F32, tag="invw2")
            nc.scalar.activation(invw2[:], rec2[:], AF.Sqrt)
            # reshuffle to row-major invw[r, t] (4 partition-shift DMAs), then
            # per-j transposes so partition p carries t = NT*p + j, matching
            # the strided lhsT slices (contiguous 8KB/partition output stores)
            invw = prep.tile([rows, T], F32, tag="invw")
            for c in range(CH):
                nc.scalar.dma_start(
                    invw[:, c * CL : (c + 1) * CL],
                    invw2[c * rows : (c + 1) * rows, :],
                )
            invwv = invw[:].rearrange("r (p j) -> r j p", j=NT)
            invT3 = constp.tile([128, NT * rows], F32)
            for j in range(NT):
                ps = prep_ps.tile([128, 512], F32, tag="ps")
                nc.tensor.transpose(
                    ps[0:128, 0:rows], invwv[:, j, :], ident[0:rows, 0:rows]
                )
                copy_ps(invT3[:, j * rows : (j + 1) * rows], ps[0:128, 0:rows], rows)

            def sc_ap(row, j):
                # scale column for (row, t-tile j): partition p <-> t = NT*p+j
                col = j * rows + row
                return invT3[:, col : col + 1]

            # ---- main loop: pairs of rows on the two PE tile halves ----
            def evict(dst, ps, sc):
                if eng_ns[0] + ACT_EVICT_NS <= eng_ns[1] + DVE_EVICT_NS:
                    nc.scalar.activation(dst, ps, AF.Relu, scale=sc)
                    eng_ns[0] += ACT_EVICT_NS
                else:
                    nc.vector.tensor_scalar(
                        dst, ps, sc, 0.0, op0=ALU.mult, op1=ALU.max
                    )
                    eng_ns[1] += DVE_EVICT_NS

            ps_slot = [0]

            def ps_tile():
                k = ps_slot[0] = (ps_slot[0] + 1) % 4
                if k == 0:
                    pt = prep_ps.tile([128, 512], F32, tag="ps")
                    return pt[:, 0:N]
                pt = mmps.tile([128, N], F32, tag="mm")
                return pt[:]

            for i in range(rows // 2):
                r0, r1 = 2 * i, 2 * i + 1
                c0, c1 = r0 % C, r1 % C
                # row r0's windows on partitions 0-63, r1's on 64-127
                xw = xwp.tile([128, T], win_dt)
                nc.sync.dma_start(
                    xw[0:64, :], bass.AP(win_src, r0 * TP, [[1, 64], [1, T]])
                )
                nc.sync.dma_start(
                    xw[64:128, :], bass.AP(win_src, r1 * TP, [[1, 64], [1, T]])
                )
                xw_mm = xw[:] if use16 else xw[:].bitcast(mm_dtype)
                # weight columns strided by NT: output partition p <-> t = NT*p+j
                xwv = xw_mm.rearrange("p (t j) -> p j t", j=NT)
                ostA = ostagep.tile([128, NT, N], out_dt, tag="oA")
                ostB = ostagep.tile([128, NT, N], out_dt, tag="oB")
                for j in range(NT):
                    psA = ps_tile()
                    nc.tensor.matmul(
                        psA, xwv[0:64, j, :], snT[0:64, c0 * N : (c0 + 1) * N],
                        start=True, stop=True,
                    )
                    psB = ps_tile()
                    nc.tensor.matmul(
                        psB, xwv[64:128, j, :], snT[64:128, c1 * N : (c1 + 1) * N],
                        start=True, stop=True,
                    )
                    evict(ostA[:, j, :], psA, sc_ap(r0, j))
                    evict(ostB[:, j, :], psB, sc_ap(r1, j))
                oA = out.ap()[r0].rearrange("(p j) n -> p j n", j=NT)
                oB = out.ap()[r1].rearrange("(p j) n -> p j n", j=NT)
                h = NT // 2
                nc.gpsimd.dma_start(oA[:, 0:h, :], ostA[:, 0:h, :])
                nc.sync.dma_start(oB[:, 0:h, :], ostB[:, 0:h, :])
                nc.gpsimd.dma_start(oA[:, h:NT, :], ostA[:, h:NT, :])
                nc.sync.dma_start(oB[:, h:NT, :], ostB[:, h:NT, :])
    _split_matmul_waits(nc)
    return nc


def _split_matmul_waits(nc):
    """This walrus build accepts only ONE sync wait per instruction (Matmult
    LDWEIGHTS slot, Activation, ...).  Move extra waits onto nops inserted
    just before the instruction on the same engine."""
    for f in nc.m.functions:
        for bb in f.blocks:
            out = []
            for inst in bb.instructions:
                if (
                    inst.sync_info is not None
                    and len(inst.sync_info.on_wait) > 1
                ):
                    waits = list(inst.sync_info.on_wait)
                    for w in waits[:-1]:
                        nop = mybir.InstNoOp(
                            name=nc.get_next_instruction_name(), ins=[], outs=[]
                        )
                        nop.engine = inst.engine
                        nop.sync_info = mybir.SyncInfo(on_wait=[w], on_update=[])
                        out.append(nop)
                    inst.sync_info = mybir.SyncInfo(
                        on_wait=[waits[-1]], on_update=list(inst.sync_info.on_update)
                    )
                out.append(inst)
            bb.instructions = out


def _permute_shapelets(shp):
    """[C, N, S] -> [128, C*NK, S] so chunk i's transpose lands columns
    i*128..(i+1)*128 of snT in original (c, n) order."""
    flat = shp.reshape(C * N, S)  # row R = c*N + n
    return np.ascontiguousarray(
        flat.reshape(NCK, 128, S).transpose(1, 0, 2)
    )  # [p, i, s] = flat[i*128 + p]


def _shard_inputs(x, shapelets, rows_per_core):
    xpad = np.pad(
        np.asarray(x, dtype=np.float32), ((0, 0), (0, 0), (PAD_L, PAD_R))
    )  # [B, C, TP]
    shp = _permute_shapelets(np.asarray(shapelets, dtype=np.float32))
    bpc = rows_per_core // C
    in_maps = []
    for core in range(NCORES):
        xs = xpad[core * bpc : (core + 1) * bpc].reshape(rows_per_core, TP)
        in_maps.append({"xp": np.ascontiguousarray(xs), "shp": shp})
    return in_maps


def _install_ntff_shim():
    """The image's antenv lacks axon_hooks; synthesize it so trace=True works."""
    import types

    if "antenv.axon_hooks" in sys.modules:
        return
    try:
        import antenv
        from trn_agent_boot.trn_boot import _ntff_profile_via_ctypes
    except ImportError:
        return
    mod = types.ModuleType("antenv.axon_hooks")
    state = {"hook": None}
    mod.set_axon_ntff_profile_hook = lambda h: state.__setitem__("hook", h)
    mod.get_axon_ntff_profile_hook = lambda: state["hook"]
    sys.modules["antenv.axon_hooks"] = mod
    antenv.axon_hooks = mod
    try:
        mod.set_axon_ntff_profile_hook(
            _ntff_profile_via_ctypes("/opt/axon/libaxon_pjrt.so")
        )
    except OSError:
        pass


def kernel(x, shapelets, trace=False, mm_dtype=F16, out_np_dtype=np.float16):
    if trace:
        _install_ntff_shim()
    rows = B * C // NCORES
    nc = build_nc(rows=rows, mm_dtype=mm_dtype, out_np_dtype=out_np_dtype)
    in_maps = _shard_inputs(x, shapelets, rows)
    res = run_bass_kernel_spmd(
        nc, in_maps, core_ids=list(range(NCORES)), trace=trace
    )
    bpc = rows // C
    outs = [r["out"].reshape(bpc, C, T, N) for r in res.results]
    full = np.concatenate(outs, axis=0)
    if full.dtype != np.float32:
        full = full.astype(np.float32)
    if trace:
        kernel.last_results = res
    return full


kernel.last_results = None


# revision 24
# speedup vs baseline: 1.1093x; 1.1093x over previous
"""Trainium2 Bass kernel for MaxCosineSimilarityBlock.

Reference computation (per batch b, channel c):
  windows  xw[t, s] = xpad[t + s]          (xpad = x padded by 31/32 zeros, S=64)
  xn[t, :] = xw[t, :] / max(||xw[t, :]||, 1e-8)
  sn[n, :] = shapelets[c, n, :] / max(||shapelets[c, n, :]||, 1e-8)
  out[b, c, t, n] = relu(xn[t, :] @ sn[n, :])

Shapes: x [32, 8, 1024] f32, shapelets [8, 512, 64] f32 -> out [32, 8, 1024, 512] f32.

Strategy: data-parallel over batch B across 8 cores (4 batches/core = 32
(b, c) rows/core).  Per row the conv-as-matmul runs on the PE with the
im2col window matrix streamed via an overlapping access pattern from a
fp16 copy of the padded rows staged in DRAM scratch (halves the 64x read
amplification of the im2col load).  Weight columns are strided by NT so
output partition p carries t = NT*p + j, which makes each partition's
output staging block 8KB-contiguous in HBM (fast stores).

The S=64 contraction only fills half the 128-row PE array, so rows are
processed in PAIRS mapped to the two 64-row PE tile groups
(tile_position (0,0) / (64,0)): row 2i's windows sit on SBUF partitions
0-63, row 2i+1's on partitions 64-127, the normalized+transposed
shapelets snT are mirrored across both partition halves once, and the two
matmuls per t-tile execute concurrently on the PE (~2x throughput at
K=64).

  lhsT = XwinT [S=64, 128 t]  (fp16 weights, FWL-accelerated self-load)
  rhs  = snT   [S=64, N=512]  (normalized shapelets, transposed+cast once)
  psum [128 t, 512 n] f32

PSUM eviction applies relu and the per-window inverse norm (per-partition
scale) while downcasting to the f16 output staging tile; evictions
alternate between the ACT and DVE engines (neither alone can keep up) and
are the steady-state governor, so prep is kept off their critical path:
shapelets arrive host-permuted for a contiguous load, their norm chain is
batched per channel-group on DVE, the window-norm log-tree runs on the
otherwise idle GPSIMD engine, and output stores ride the GPSIMD (SWDGE)
queue.
"""

import os
import sys

for _p in ("/opt/trn_rl_repo", "/root/.axon_site/_ro/trn_rl_repo"):
    if os.path.isdir(_p) and _p not in sys.path:
        sys.path.insert(0, _p)

import numpy as np

import concourse.bass as bass
import concourse.mybir as mybir
from concourse import masks, tile
from concourse.bass_utils import run_bass_kernel_spmd

F32 = mybir.dt.float32
F32R = mybir.dt.float32r
F16 = mybir.dt.float16
BF16 = mybir.dt.bfloat16
AF = mybir.ActivationFunctionType
ALU = mybir.AluOpType

B, C, T, S, N = 32, 8, 1024, 64, 512
NCORES = 8
PAD_L, PAD_R = (S - 1) // 2, (S - 1) // 2 + (S - 1) % 2  # 31, 32
TP = T + S - 1  # 1087
NT = T // 128  # 8 t-tiles per row
NK = N // 128  # 4 shapelet chunks per channel
NCK = C * NK  # 32 shapelet chunks total

# eviction cost model (ns, trace-fit) for load balancing ACT vs DVE
ACT_EVICT_NS = 737.0
DVE_EVICT_NS = 744.0

# window-norm chunking: each row's sliding sums computed as CH chunks of CL
CH, CL = 4, 256
CW = CL + S - 1  # 319 elements feed one chunk's sums


def build_nc(rows=B * C // NCORES, mm_dtype=F16, out_np_dtype=np.float16):
    """Build the per-core Bass program. `rows` = number of (b, c) rows."""
    out_dt = mybir.dt.from_np(np.dtype(out_np_dtype))
    use16 = mm_dtype in (F16, BF16)
    td = mm_dtype if use16 else F32  # transpose/snT dtype
    nc = bass.Bass("TRN2", target_bir_lowering=False, debug=False)
    xp = nc.dram_tensor("xp", [rows, TP], F32, kind="ExternalInput")
    # host sends shapelets pre-permuted: shp[p, i, s] = orig[(i*128 + p)//N,
    # (i*128 + p) % N, s] so a contiguous load puts chunk columns in order
    shp = nc.dram_tensor("shp", [128, NCK, S], F32, kind="ExternalInput")
    out = nc.dram_tensor("out", [rows, T, N], out_dt, kind="ExternalOutput")
    if use16:
        xp16 = nc.dram_tensor("xp16", [rows, TP], mm_dtype, kind="Internal")

    with tile.TileContext(nc) as tc:
        with (
            tc.tile_pool(name="const", bufs=1) as constp,
            tc.tile_pool(name="prep", bufs=2) as prep,
            tc.tile_pool(name="prep_ps", bufs=2, space="PSUM") as prep_ps,
            tc.tile_pool(name="xw", bufs=3) as xwp,
            tc.tile_pool(name="ostage", bufs=4) as ostagep,
            tc.tile_pool(name="mm_ps", bufs=6, space="PSUM") as mmps,
        ):
            ident = constp.tile([128, 128], F32)
            masks.make_identity(nc, ident[:])

            # ---- input loads (sync ring) + fp16 DRAM scratch staging ----
            if use16:
                xr = prep.tile([rows, TP], F32, tag="xr")
                nc.sync.dma_start(xr[:], xp.ap())
                xr16 = prep.tile([rows, TP], mm_dtype, tag="xr16")
                nc.vector.tensor_copy(xr16[:], xr[:])
                win_src, win_dt = xp16, mm_dtype
            else:
                win_src, win_dt = xp, F32
            sh_sb = prep.tile([128, NCK, S], F32, tag="shload")
            nc.sync.dma_start(sh_sb[:], shp.ap())
            # chunked overlapped rows for the 128-partition norm chain:
            # partition c*rows + r holds xpad[r, CL*c : CL*c + CW]
            xr2 = prep.tile([rows * CH, 320], F32, tag="xr2")
            nc.sync.dma_start(
                xr2[:, 0:CW], bass.AP(xp, 0, [[CL, CH], [TP, rows], [1, CW]])
            )
            if use16:
                nc.sync.dma_start(xp16.ap(), xr16[:])

            if use16:
                ident_t = constp.tile([128, 128], td)
                nc.scalar.copy(ident_t[:], ident[:])
            else:
                ident_t = ident

            # engine chooser: balance ACT vs DVE by accumulated busy-ns
            eng_ns = [0.0, 0.0]  # (ACT, DVE)

            def copy_ps(dst, src, fd):
                """PSUM->SBUF copy on the less-loaded of ACT/DVE."""
                act_c, dve_c = (172 + fd) / 1.2 + 100, (120 + fd) / 0.96 + 100
                if eng_ns[0] + act_c <= eng_ns[1] + dve_c:
                    nc.scalar.copy(dst, src)
                    eng_ns[0] += act_c
                else:
                    nc.vector.tensor_copy(dst, src)
                    eng_ns[1] += dve_c

            # ---- shapelet prep: per-group batched inverse norms (DVE), ----
            # ---- fp16 PE transposes into snT's lower partition half    ----
            snT = constp.tile([128, NCK * 128], td)
            GN, GS = 4, NCK // 4  # channel groups of 8 chunks
            sq = prep.tile([128, NCK * S], F32, tag="sq")
            ssq = prep.tile([128, NCK, 1], F32, tag="ssq")
            recs = prep.tile([128, NCK], F32, tag="recs")
            invs = prep.tile([128, NCK], F32, tag="invs")
            nrm16 = prep.tile([128, NCK, S], td, tag="nrm16")
            shv = sh_sb[:].rearrange("p k s -> p (k s)")
            ssqf = ssq[:].rearrange("p k 1 -> p (k 1)")
            for g in range(GN):
                ck0, ck1 = g * GS, (g + 1) * GS
                fsl = slice(ck0 * S, ck1 * S)
                nc.vector.tensor_mul(sq[:, fsl], shv[:, fsl], shv[:, fsl])
                nc.vector.reduce_sum(
                    ssq[:, ck0:ck1, :],
                    sq[:, fsl].rearrange("p (k s) -> p k s", s=S),
                    axis=mybir.AxisListType.X,
                )
                nc.vector.tensor_scalar_max(
                    ssqf[:, ck0:ck1], ssqf[:, ck0:ck1], 1e-16
                )
                nc.vector.reciprocal(recs[:, ck0:ck1], ssqf[:, ck0:ck1])
                nc.scalar.activation(
                    invs[:, ck0:ck1], recs[:, ck0:ck1], AF.Sqrt
                )
                nc.vector.tensor_mul(
                    nrm16[:, ck0:ck1, :],
                    sh_sb[:, ck0:ck1, :],
                    invs[:, ck0:ck1][:, :, None].broadcast_to([128, GS, S]),
                )
            tdw = 64 if use16 else 128  # f32-element width of a [64,128] td tile
            for ck in range(NCK):
                ps = prep_ps.tile([128, 512], F32, tag="ps")
                ps_t = ps[0:64, 0:tdw].bitcast(td)
                nc.tensor.transpose(ps_t, nrm16[:, ck, :], ident_t[:])
                copy_ps(snT[0:64, ck * 128 : (ck + 1) * 128], ps_t, 128)
            # mirror to partitions 64-127 for the second PE tile group
            nc.gpsimd.dma_start(snT[64:128, :], snT[0:64, :])

            # ---- window inverse norms (gpsimd log-tree, 128 partitions) ----
            cur = prep.tile([rows * CH, 320], F32, tag="wc0")
            nc.gpsimd.tensor_mul(cur[:, 0:CW], xr2[:, 0:CW], xr2[:, 0:CW])
            for k in range(6):
                off = 1 << k
                ln = CW - (2 * off - 1)
                nxt = prep.tile([rows * CH, 320], F32, tag=f"wc{k + 1}")
                nc.gpsimd.tensor_add(
                    nxt[:, 0:ln], cur[:, 0:ln], cur[:, off : off + ln]
                )
                cur = nxt
            # cur[:, 0:CL] = sliding sums; invw2[c*rows+r, q] = 1/||win(CL*c+q)||
            nc.gpsimd.tensor_scalar_max(cur[:, 0:CL], cur[:, 0:CL], 1e-16)
            rec2 = prep.tile([rows * CH, CL], F32, tag="rec2")
            nc.vector.reciprocal(rec2[:], cur[:, 0:CL])
            invw2 = prep.tile([rows * CH, CL], F32, tag="invw2")
            nc.scalar.activation(invw2[:], rec2[:], AF.Sqrt)
            # reshuffle to row-major invw[r, t] (4 partition-shift DMAs), then
            # per-j transposes so partition p carries t = NT*p + j, matching
            # the strided lhsT slices (contiguous 8KB/partition output stores)
            invw = prep.tile([rows, T], F32, tag="invw")
            for c in range(CH):
                nc.scalar.dma_start(
                    invw[:, c * CL : (c + 1) * CL],
                    invw2[c * rows : (c + 1) * rows, :],
                )
            invwv = invw[:].rearrange("r (p j) -> r j p", j=NT)
            invT3 = constp.tile([128, NT * rows], F32)
            for j in range(NT):
                ps = prep_ps.tile([128, 512], F32, tag="ps")
                nc.tensor.transpose(
                    ps[0:128, 0:rows], invwv[:, j, :], ident[0:rows, 0:rows]
                )
                copy_ps(invT3[:, j * rows : (j + 1) * rows], ps[0:128, 0:rows], rows)

            def sc_ap(row, j):
                # scale column for (row, t-tile j): partition p <-> t = NT*p+j
                col = j * rows + row
                return invT3[:, col : col + 1]

            # ---- main loop: pairs of rows on the two PE tile halves ----
            def evict(dst, ps, sc):
                if eng_ns[0] + ACT_EVICT_NS <= eng_ns[1] + DVE_EVICT_NS:
                    nc.scalar.activation(dst, ps, AF.Relu, scale=sc)
                    eng_ns[0] += ACT_EVICT_NS
                else:
                    nc.vector.tensor_scalar(
                        dst, ps, sc, 0.0, op0=ALU.mult, op1=ALU.max
                    )
                    eng_ns[1] += DVE_EVICT_NS

            ps_slot = [0]

            def ps_tile():
                k = ps_slot[0] = (ps_slot[0] + 1) % 4
                if k == 0:
                    pt = prep_ps.tile([128, 512], F32, tag="ps")
                    return pt[:, 0:N]
                pt = mmps.tile([128, N], F32, tag="mm")
                return pt[:]

            for i in range(rows // 2):
                r0, r1 = 2 * i, 2 * i + 1
                c0, c1 = r0 % C, r1 % C
                # row r0's windows on partitions 0-63, r1's on 64-127
                xw = xwp.tile([128, T], win_dt)
                nc.sync.dma_start(
                    xw[0:64, :], bass.AP(win_src, r0 * TP, [[1, 64], [1, T]])
                )
                nc.sync.dma_start(
                    xw[64:128, :], bass.AP(win_src, r1 * TP, [[1, 64], [1, T]])
                )
                xw_mm = xw[:] if use16 else xw[:].bitcast(mm_dtype)
                # weight columns strided by NT: output partition p <-> t = NT*p+j
                xwv = xw_mm.rearrange("p (t j) -> p j t", j=NT)
                ostA = ostagep.tile([128, NT, N], out_dt, tag="oA")
                ostB = ostagep.tile([128, NT, N], out_dt, tag="oB")
                for j in range(NT):
                    psA = ps_tile()
                    nc.tensor.matmul(
                        psA, xwv[0:64, j, :], snT[0:64, c0 * N : (c0 + 1) * N],
                        start=True, stop=True,
                    )
                    psB = ps_tile()
                    nc.tensor.matmul(
                        psB, xwv[64:128, j, :], snT[64:128, c1 * N : (c1 + 1) * N],
                        start=True, stop=True,
                    )
                    evict(ostA[:, j, :], psA, sc_ap(r0, j))
                    evict(ostB[:, j, :], psB, sc_ap(r1, j))
                nc.gpsimd.dma_start(
                    out.ap()[r0].rearrange("(p j) n -> p j n", j=NT), ostA[:]
                )
                nc.sync.dma_start(
                    out.ap()[r1].rearrange("(p j) n -> p j n", j=NT), ostB[:]
                )
    _split_matmul_waits(nc)
    return nc


def _split_matmul_waits(nc):
    """This walrus build accepts only ONE sync wait per instruction (Matmult
    LDWEIGHTS slot, Activation, ...).  Move extra waits onto nops inserted
    just before the instruction on the same engine."""
    for f in nc.m.functions:
        for bb in f.blocks:
            out = []
            for inst in bb.instructions:
                if (
                    inst.sync_info is not None
                    and len(inst.sync_info.on_wait) > 1
                ):
                    waits = list(inst.sync_info.on_wait)
                    for w in waits[:-1]:
                        nop = mybir.InstNoOp(
                            name=nc.get_next_instruction_name(), ins=[], outs=[]
                        )
                        nop.engine = inst.engine
                        nop.sync_info = mybir.SyncInfo(on_wait=[w], on_update=[])
                        out.append(nop)
                    inst.sync_info = mybir.SyncInfo(
                        on_wait=[waits[-1]], on_update=list(inst.sync_info.on_update)
                    )
                out.append(inst)
            bb.instructions = out


def _permute_shapelets(shp):
    """[C, N, S] -> [128, C*NK, S] so chunk i's transpose lands columns
    i*128..(i+1)*128 of snT in original (c, n) order."""
    flat = shp.reshape(C * N, S)  # row R = c*N + n
    return np.ascontiguousarray(
        flat.reshape(NCK, 128, S).transpose(1, 0, 2)
    )  # [p, i, s] = flat[i*128 + p]


def _shard_inputs(x, shapelets, rows_per_core):
    xpad = np.pad(
        np.asarray(x, dtype=np.float32), ((0, 0), (0, 0), (PAD_L, PAD_R))
    )  # [B, C, TP]
    shp = _permute_shapelets(np.asarray(shapelets, dtype=np.float32))
    bpc = rows_per_core // C
    in_maps = []
    for core in range(NCORES):
        xs = xpad[core * bpc : (core + 1) * bpc].reshape(rows_per_core, TP)
        in_maps.append({"xp": np.ascontiguousarray(xs), "shp": shp})
    return in_maps


def _install_ntff_shim():
    """The image's antenv lacks axon_hooks; synthesize it so trace=True works."""
    import types

    if "antenv.axon_hooks" in sys.modules:
        return
    try:
        import antenv
        from trn_agent_boot.trn_boot import _ntff_profile_via_ctypes
    except ImportError:
        return
    mod = types.ModuleType("antenv.axon_hooks")
    state = {"hook": None}
    mod.set_axon_ntff_profile_hook = lambda h: state.__setitem__("hook", h)
    mod.get_axon_ntff_profile_hook = lambda: state["hook"]
    sys.modules["antenv.axon_hooks"] = mod
    antenv.axon_hooks = mod
    try:
        mod.set_axon_ntff_profile_hook(
            _ntff_profile_via_ctypes("/opt/axon/libaxon_pjrt.so")
        )
    except OSError:
        pass


def kernel(x, shapelets, trace=False, mm_dtype=F16, out_np_dtype=np.float16):
    if trace:
        _install_ntff_shim()
    rows = B * C // NCORES
    nc = build_nc(rows=rows, mm_dtype=mm_dtype, out_np_dtype=out_np_dtype)
    in_maps = _shard_inputs(x, shapelets, rows)
    res = run_bass_kernel_spmd(
        nc, in_maps, core_ids=list(range(NCORES)), trace=trace
    )
    bpc = rows // C
    outs = [r["out"].reshape(bpc, C, T, N) for r in res.results]
    full = np.concatenate(outs, axis=0)
    if full.dtype != np.float32:
        full = full.astype(np.float32)
    if trace:
        kernel.last_results = res
    return full


kernel.last_results = None
